# revision 20
# baseline (speedup 1.0000x reference)
"""Trainium2 Bass kernel for nn_MixerLaCT (LaCT fast-weight mixer).

Sharding: data-parallel over (batch, head) = 8 shards on 8 NeuronCores.
Each core computes its head's q/k/v projections (the full-D rms_norm scale
cancels inside the per-head l2-normalize up to the 1e-5 epsilon, so heads
are independent; the rms norm weights fold into the projection matrix on
the host), then runs the sequential 4-chunk fast-weight loop locally.

All matmuls run as float32r (full-rate fp32 streaming mode, ~1.5e-4 rel).
"""
import os
import numpy as np
from contextlib import ExitStack

import concourse.bass as bass
from concourse import bacc
import concourse.mybir as mybir
import concourse.tile as tile
from concourse.bass import ds, ts
from concourse.bass_utils import run_bass_kernel_spmd

f32 = mybir.dt.float32
f32r = mybir.dt.float32r
AF = mybir.ActivationFunctionType
ALU = mybir.AluOpType

B, S, D = 2, 8192, 1024
NH, DH, DHID = 4, 256, 512
CHUNK = 2048
BASE_LR_INV = float(np.log(np.expm1(1e-3)))
NS_COEFFS = ((4.0848, -6.8946, 2.927), (3.9505, -6.3029, 2.6377),
             (3.7418, -5.5913, 2.3037), (2.8769, -3.1427, 1.2046),
             (2.8366, -3.0525, 1.2012))
NCORES = 8
WCOLS = 772            # q(256) | k(256) | v(256) | lr(3) | pad(1)


def build_nc(s_total=S):
    n_chunks = s_total // CHUNK
    nc = bacc.Bacc(None, target_bir_lowering=False, debug=False)

    hidden = nc.declare_dram_parameter("hidden", [s_total, D], f32r, isOutput=False)
    wcat_d = nc.declare_dram_parameter("wcat", [D, WCOLS], f32r, isOutput=False)
    lrb_d = nc.declare_dram_parameter("lrb", [1, 3], f32, isOutput=False)
    w0_d = nc.declare_dram_parameter("w0", [DHID, DH], f32r, isOutput=False)
    w1_d = nc.declare_dram_parameter("w1", [DH, DHID], f32r, isOutput=False)
    w2_d = nc.declare_dram_parameter("w2", [DHID, DH], f32r, isOutput=False)
    ident_d = nc.declare_dram_parameter("ident", [128, 128], f32r, isOutput=False)
    ones_c = nc.declare_dram_parameter("ones_c", [128, 1], f32, isOutput=False)
    ones_r = nc.declare_dram_parameter("ones_r", [1, 128], f32, isOutput=False)
    out_d = nc.declare_dram_parameter("out", [s_total, DH], f32, isOutput=True)

    hid_r = hidden.rearrange("(g p) d -> p g d", p=128)       # [128, s/128, 1024]
    out_r = out_d.rearrange("(g p) d -> p g d", p=128)        # [128, s/128, 256]
    wcat_r = wcat_d.rearrange("(kt p) n -> p kt n", p=128)    # [128, 8, 772]

    with tile.TileContext(nc) as tc:
        with nc.allow_low_precision(reason="float32r rounding is intended"), ExitStack() as ctx:
            single = ctx.enter_context(tc.tile_pool(name="single", bufs=1))
            state = ctx.enter_context(tc.tile_pool(name="state", bufs=1))
            chunkb = ctx.enter_context(tc.tile_pool(name="chunkb", bufs=1))
            work = ctx.enter_context(tc.tile_pool(name="work", bufs=2))
            nsp = ctx.enter_context(tc.tile_pool(name="nsp", bufs=2))
            ns1 = ctx.enter_context(tc.tile_pool(name="ns1", bufs=1))
            small = ctx.enter_context(tc.tile_pool(name="small", bufs=2))
            psA = ctx.enter_context(tc.tile_pool(name="psA", bufs=2, space="PSUM"))
            psB = ctx.enter_context(tc.tile_pool(name="psB", bufs=3, space="PSUM"))
            psT = ctx.enter_context(tc.tile_pool(name="psT", bufs=1, space="PSUM"))

            def transpose_blocks(blocks):
                """blocks: list of (src [128,128] AP, dst [128,128] AP).
                PE-transpose in groups of 4 through one psum bank."""
                for i in range(0, len(blocks), 4):
                    grp = blocks[i:i + 4]
                    trp = psT.tile([128, 512], f32r, tag="ptr")
                    for j, (src, _dst) in enumerate(grp):
                        nc.tensor.transpose(trp[:, ds(j * 128, 128)], src, ident)
                    for j, (_src, dst) in enumerate(grp):
                        nc.vector.tensor_copy(dst, trp[:, ds(j * 128, 128)])

            # ---- constants ----
            ident = single.tile([128, 128], f32r)
            nc.sync.dma_start(out=ident, in_=ident_d[:, :])
            onc = single.tile([128, 1], f32)
            nc.sync.dma_start(out=onc, in_=ones_c[:, :])
            onr = single.tile([1, 128], f32)
            nc.sync.dma_start(out=onr, in_=ones_r[:, :])
            lrb_sb = single.tile([128, 3], f32)
            lrb_ap = lrb_d[:, :]
            lrb_b = bass.AP(tensor=lrb_ap.tensor, offset=lrb_ap.offset,
                            ap=[[0, 128], [1, 3]])
            nc.sync.dma_start(out=lrb_sb, in_=lrb_b)
            wcat = single.tile([128, 8, WCOLS], f32r)
            nc.sync.dma_start(out=wcat, in_=wcat_r)

            # ---- fast-weight state ----
            w0n = state.tile([128, 4, 256], f32r)   # [512, 256] natural
            w1n = state.tile([128, 2, 512], f32r)   # [256, 512] natural
            w2n = state.tile([128, 4, 256], f32r)
            w0t = state.tile([128, 2, 512], f32r)   # transposed layouts
            w1t = state.tile([128, 4, 256], f32r)
            w2t = state.tile([128, 2, 512], f32r)
            nc.sync.dma_start(out=w0n, in_=w0_d.rearrange("(mt p) d -> p mt d", p=128))
            nc.sync.dma_start(out=w1n, in_=w1_d.rearrange("(mt p) d -> p mt d", p=128))
            nc.sync.dma_start(out=w2n, in_=w2_d.rearrange("(mt p) d -> p mt d", p=128))

            def build_T(wn, wt, nmt):
                # wn [128, nmt, F] (rows=nmt*128) -> wt [128, F/128, nmt*128]
                blocks = []
                for dt_ in range(wn.shape[2] // 128):
                    for mt in range(nmt):
                        blocks.append((wn[:, mt, ds(dt_ * 128, 128)],
                                       wt[:, dt_, ds(mt * 128, 128)]))
                transpose_blocks(blocks)

            build_T(w0n, w0t, 4)
            build_T(w1n, w1t, 2)
            build_T(w2n, w2t, 4)

            # initial row norms (before any update), per natural-layout row
            n0 = single.tile([128, 4], f32)
            n1 = single.tile([128, 2], f32)
            n2 = single.tile([128, 4], f32)
            for (wn, nn, nmt) in ((w0n, n0, 4), (w1n, n1, 2), (w2n, n2, 4)):
                for mt in range(nmt):
                    junk = work.tile([128, 1024], f32, tag="junkw")
                    nc.scalar.activation(out=junk[:, 0:wn.shape[2]],
                                         in_=wn[:, mt, :].bitcast(f32), func=AF.Square,
                                         accum_out=nn[:, mt:mt + 1])
                nc.scalar.activation(out=nn, in_=nn, func=AF.Sqrt)

            # ---- per-chunk buffers ----
            qn = chunkb.tile([128, 8, 256], f32r)
            kn = chunkb.tile([128, 8, 256], f32r)
            vn = chunkb.tile([128, 8, 256], f32r)
            oc = chunkb.tile([128, 8, 256], f32)
            lrc = chunkb.tile([128, 8, 3], f32)
            lrr = chunkb.tile([128, 8, 3], f32)
            ssqk = chunkb.tile([128, 16], f32)
            dw0a = chunkb.tile([128, 4, 256], f32)
            dw1a = chunkb.tile([128, 2, 512], f32)
            dw2a = chunkb.tile([128, 4, 256], f32)

            def phase1(gtb_base):
                """project 8 token-blocks (1024 tokens) into qn/kn/vn/lrc.

                ACT table discipline: the per-tblock loop only uses Square
                and Copy (present in every table); Sqrt and Exp/Ln are
                batched at the end so the chunk does ~3 table swaps."""
                for tb in range(8):
                    gtb = gtb_base + tb
                    hid_t = work.tile([128, 1024], f32r, tag="hid")
                    nc.sync.dma_start(out=hid_t, in_=hid_r[:, gtb, :])
                    hT = work.tile([128, 8, 128], f32r, tag="hT")
                    transpose_blocks([(hid_t[:, ds(dt_ * 128, 128)], hT[:, dt_, :])
                                      for dt_ in range(8)])

                    qk_ps = psA.tile([128, 512], f32, tag="pA")
                    vl_ps = psA.tile([128, 260], f32, tag="pA")
                    for kt in range(8):
                        nc.tensor.matmul(qk_ps, hT[:, kt, :], wcat[:, kt, 0:512],
                                         start=(kt == 0), stop=(kt == 7))
                    for kt in range(8):
                        nc.tensor.matmul(vl_ps, hT[:, kt, :], wcat[:, kt, 512:772],
                                         start=(kt == 0), stop=(kt == 7))

                    for (dst, lo, col) in ((qn, 0, tb), (kn, 256, 8 + tb)):
                        junk = work.tile([128, 1024], f32, tag="junkw")
                        nc.scalar.activation(out=junk[:, 0:256], in_=qk_ps[:, ds(lo, 256)],
                                             func=AF.Square,
                                             accum_out=ssqk[:, col:col + 1])
                        nc.scalar.copy(dst[:, tb, :], qk_ps[:, ds(lo, 256)])
                    nc.scalar.copy(vn[:, tb, :], vl_ps[:, 0:256])
                    nc.vector.tensor_add(lrr[:, tb, :], vl_ps[:, 256:259], lrb_sb)

                # softplus(x) = ln(1 + exp(x)) — exp/ln share one table
                lrf = lrr.rearrange("p a b -> p (a b)")
                exj = work.tile([128, 24], f32, tag="exj")
                nc.scalar.activation(out=exj, in_=lrf, func=AF.Exp)
                nc.vector.tensor_scalar_add(exj, exj, 1.0)
                nc.scalar.activation(out=lrc.rearrange("p a b -> p (a b)"), in_=exj,
                                     func=AF.Ln)
                # l2-norm scales: 1/(sqrt(ss)+1e-5), batched
                sc = small.tile([128, 16], f32, tag="sc")
                nc.scalar.activation(out=sc, in_=ssqk, func=AF.Sqrt)
                nc.vector.tensor_scalar_add(sc, sc, 1e-5)
                nc.vector.reciprocal(sc, sc)
                for tb in range(8):
                    nc.gpsimd.tensor_scalar_mul(qn[:, tb, :], qn[:, tb, :].bitcast(f32),
                                                sc[:, tb:tb + 1])
                    nc.gpsimd.tensor_scalar_mul(kn[:, tb, :], kn[:, tb, :].bitcast(f32),
                                                sc[:, 8 + tb:9 + tb])

            def phase2a(gtb_base):
                """apply current fast weights to queries (4 slabs of 256 tokens)."""
                for sl4 in range(4):
                    tb0 = sl4 * 2
                    qT = work.tile([128, 2, 256], f32r, tag="qT")
                    blocks = []
                    for dt_ in range(2):
                        for j in range(2):
                            blocks.append((qn[:, tb0 + j, ds(dt_ * 128, 128)],
                                           qT[:, dt_, ds(j * 128, 128)]))
                    transpose_blocks(blocks)

                    gq_ps = psA.tile([128, 4, 256], f32, tag="pA")
                    hq_ps = psA.tile([128, 4, 256], f32, tag="pA")
                    for mt in range(4):
                        for kt in range(2):
                            nc.tensor.matmul(gq_ps[:, mt, :],
                                             w0t[:, kt, ds(mt * 128, 128)], qT[:, kt, :],
                                             start=(kt == 0), stop=(kt == 1))
                    for mt in range(4):
                        for kt in range(2):
                            nc.tensor.matmul(hq_ps[:, mt, :],
                                             w2t[:, kt, ds(mt * 128, 128)], qT[:, kt, :],
                                             start=(kt == 0), stop=(kt == 1))
                    sgq = work.tile([128, 4, 256], f32, tag="sgq")
                    nc.scalar.activation(out=sgq, in_=gq_ps, func=AF.Sigmoid)
                    tq = work.tile([128, 4, 256], f32, tag="tq")
                    nc.vector.tensor_mul(tq, sgq, gq_ps)
                    gh = work.tile([128, 4, 256], f32r, tag="gh")
                    nc.vector.tensor_mul(gh, tq, hq_ps)
                    for cm in range(2):
                        o_ps = psB.tile([128, 256], f32, tag="pB")
                        for kth in range(4):
                            nc.tensor.matmul(o_ps, gh[:, kth, ds(cm * 128, 128)],
                                             w1t[:, kth, :],
                                             start=(kth == 0), stop=(kth == 3))
                        nc.scalar.copy(oc[:, tb0 + cm, :], o_ps)

            def phase2b():
                """swiglu fwd/bwd on keys; accumulate dw (8 slabs of 128 tokens)."""
                for j in range(8):
                    kT = work.tile([128, 2, 128], f32r, tag="kT")
                    vT = work.tile([128, 2, 128], f32r, tag="vT")
                    blocks = [(kn[:, j, ds(dt_ * 128, 128)], kT[:, dt_, :])
                              for dt_ in range(2)]
                    blocks += [(vn[:, j, ds(dt_ * 128, 128)], vT[:, dt_, :])
                               for dt_ in range(2)]
                    transpose_blocks(blocks)

                    gba_ps = psB.tile([128, 512], f32, tag="pB")
                    hbm_ps = psB.tile([128, 512], f32, tag="pB")
                    dh_ps = psB.tile([128, 512], f32, tag="pB")
                    for kt in range(2):
                        nc.tensor.matmul(gba_ps, kT[:, kt, :], w0t[:, kt, :],
                                         start=(kt == 0), stop=(kt == 1))
                    for kt in range(2):
                        nc.tensor.matmul(hbm_ps, kT[:, kt, :], w2t[:, kt, :],
                                         start=(kt == 0), stop=(kt == 1))
                    for kt in range(2):
                        nc.tensor.matmul(dh_ps, vT[:, kt, :], w1n[:, kt, :],
                                         start=(kt == 0), stop=(kt == 1))

                    l0 = lrc[:, j, 0:1]
                    l1 = lrc[:, j, 1:2]
                    l2 = lrc[:, j, 2:3]
                    # sg = silu(gba) = gba*sig; silu'(x) = sig*(1 + x - silu(x))
                    sig = work.tile([128, 512], f32, tag="sig")
                    nc.scalar.activation(out=sig, in_=gba_ps, func=AF.Sigmoid)
                    sgT = work.tile([128, 512], f32, tag="sgT")
                    nc.vector.tensor_mul(sgT, sig, gba_ps)
                    u_t = work.tile([128, 512], f32, tag="u_t")
                    nc.vector.scalar_tensor_tensor(out=u_t, in0=gba_ps, scalar=1.0,
                                                   in1=sgT, op0=ALU.add,
                                                   op1=ALU.subtract)
                    sigd = work.tile([128, 512], f32, tag="sigd")
                    nc.vector.tensor_mul(sigd, sig, u_t)
                    hswl1 = work.tile([128, 512], f32r, tag="hswl1")
                    nc.vector.scalar_tensor_tensor(out=hswl1, in0=sgT, scalar=l1,
                                                   in1=hbm_ps, op0=ALU.mult, op1=ALU.mult)
                    t1 = work.tile([128, 512], f32, tag="t1")
                    nc.vector.tensor_mul(t1, sigd, dh_ps)
                    dgbal0 = work.tile([128, 512], f32r, tag="dgbal0")
                    nc.vector.scalar_tensor_tensor(out=dgbal0, in0=t1, scalar=l0,
                                                   in1=hbm_ps, op0=ALU.mult, op1=ALU.mult)
                    dhbml2 = work.tile([128, 512], f32r, tag="dhbml2")
                    nc.vector.scalar_tensor_tensor(out=dhbml2, in0=sgT, scalar=l2,
                                                   in1=dh_ps, op0=ALU.mult, op1=ALU.mult)

                    dw0_ps = psA.tile([128, 4, 256], f32, tag="pA")
                    for mt in range(4):
                        nc.tensor.matmul(dw0_ps[:, mt, :], dgbal0[:, ds(mt * 128, 128)],
                                         kn[:, j, :], start=True, stop=True)
                    nc.vector.tensor_add(dw0a, dw0a, dw0_ps)
                    dw1_ps = psA.tile([128, 2, 512], f32, tag="pA")
                    for mt in range(2):
                        nc.tensor.matmul(dw1_ps[:, mt, :], vn[:, j, ds(mt * 128, 128)],
                                         hswl1, start=True, stop=True)
                    nc.vector.tensor_add(dw1a, dw1a, dw1_ps)
                    dw2_ps = psA.tile([128, 4, 256], f32, tag="pA")
                    for mt in range(4):
                        nc.tensor.matmul(dw2_ps[:, mt, :], dhbml2[:, ds(mt * 128, 128)],
                                         kn[:, j, :], start=True, stop=True)
                    nc.vector.tensor_add(dw2a, dw2a, dw2_ps)

            def frob_scale(g):
                """s = 1/(||G||_F + 1e-7), broadcast to [128,1] sbuf f32."""
                nmt, nf = g.shape[1], g.shape[2]
                gf = g.rearrange("p a b -> p (a b)")
                junk = work.tile([128, 1024], f32, tag="junkw")
                pn = small.tile([128, 1], f32, tag="pn0")
                nc.scalar.activation(out=junk[:, 0:nmt * nf], in_=gf, func=AF.Square,
                                     accum_out=pn)
                ss_ps = psT.tile([128, 512], f32, tag="ptr")
                nc.tensor.matmul(ss_ps[0:1, 0:1], pn, onc, start=True, stop=True)
                sq = small.tile([1, 1], f32, tag="sq")
                nc.scalar.activation(out=sq, in_=ss_ps[0:1, 0:1], func=AF.Sqrt)
                nc.vector.tensor_scalar_add(sq, sq, 1e-7)
                nc.vector.reciprocal(sq, sq)
                bc_ps = psT.tile([128, 512], f32, tag="ptr")
                nc.tensor.matmul(bc_ps[:, 0:1], onr, sq, start=True, stop=True)
                sb = small.tile([128, 1], f32, tag="sb")
                nc.scalar.copy(sb, bc_ps[:, 0:1])
                return sb

            def newton_schulz(g, trans):
                """5-step NS orthogonalization. g [128, nmt, nf] natural layout.
                Returns the result tile in g's natural layout."""
                sb = frob_scale(g)
                if trans:   # G=[512,256]: X = G^T ([256,512]), XT = G-layout
                    xt = nsp.tile([128, 4, 256], f32r, tag="XT")
                    for mt in range(4):
                        nc.vector.tensor_scalar_mul(xt[:, mt, :], g[:, mt, :], sb)
                    x = nsp.tile([128, 2, 512], f32r, tag="X")
                    transpose_blocks([(xt[:, mt, ds(dt_ * 128, 128)],
                                       x[:, dt_, ds(mt * 128, 128)])
                                      for dt_ in range(2) for mt in range(4)])
                else:       # G=[256,512]: X = G-layout, XT = transpose
                    x = nsp.tile([128, 2, 512], f32r, tag="X")
                    for mt in range(2):
                        nc.vector.tensor_scalar_mul(x[:, mt, :], g[:, mt, :], sb)
                    xt = nsp.tile([128, 4, 256], f32r, tag="XT")
                    transpose_blocks([(x[:, mt, ds(dt_ * 128, 128)],
                                       xt[:, dt_, ds(mt * 128, 128)])
                                      for dt_ in range(4) for mt in range(2)])

                for it, (ca, cb, cc) in enumerate(NS_COEFFS):
                    last = it == len(NS_COEFFS) - 1
                    a_ps = psA.tile([128, 2, 256], f32, tag="pA")
                    for mt in range(2):
                        for kt in range(4):
                            nc.tensor.matmul(a_ps[:, mt, :],
                                             xt[:, kt, ds(mt * 128, 128)], xt[:, kt, :],
                                             start=(kt == 0), stop=(kt == 3))
                    a_sb = ns1.tile([128, 2, 256], f32r, tag="A")
                    nc.vector.tensor_copy(a_sb, a_ps)
                    a2_ps = psA.tile([128, 2, 256], f32, tag="pA")
                    for mt in range(2):
                        for kt in range(2):
                            nc.tensor.matmul(a2_ps[:, mt, :],
                                             a_sb[:, kt, ds(mt * 128, 128)], a_sb[:, kt, :],
                                             start=(kt == 0), stop=(kt == 1))
                    a2s = ns1.tile([128, 2, 256], f32, tag="A2s")
                    nc.vector.tensor_scalar_mul(a2s, a2_ps, cc)
                    bm = ns1.tile([128, 2, 256], f32r, tag="Bm")
                    nc.vector.scalar_tensor_tensor(out=bm, in0=a_sb.bitcast(f32),
                                                   scalar=cb, in1=a2s,
                                                   op0=ALU.mult, op1=ALU.add)
                    if not (last and trans):
                        y_ps = psA.tile([128, 2, 512], f32, tag="pA")
                        for mt in range(2):
                            for kt in range(2):
                                nc.tensor.matmul(y_ps[:, mt, :],
                                                 bm[:, kt, ds(mt * 128, 128)], x[:, kt, :],
                                                 start=(kt == 0), stop=(kt == 1))
                        x_new = nsp.tile([128, 2, 512], f32r, tag="X")
                        nc.vector.scalar_tensor_tensor(out=x_new, in0=x.bitcast(f32),
                                                       scalar=ca, in1=y_ps,
                                                       op0=ALU.mult, op1=ALU.add)
                    else:
                        x_new = x
                    if not (last and not trans):
                        yt_ps = psA.tile([128, 4, 256], f32, tag="pA")
                        for mt in range(4):
                            for kt in range(2):
                                nc.tensor.matmul(yt_ps[:, mt, :],
                                                 x[:, kt, ds(mt * 128, 128)], bm[:, kt, :],
                                                 start=(kt == 0), stop=(kt == 1))
                        xt_new = nsp.tile([128, 4, 256], f32r, tag="XT")
                        nc.vector.scalar_tensor_tensor(out=xt_new, in0=xt.bitcast(f32),
                                                       scalar=ca, in1=yt_ps,
                                                       op0=ALU.mult, op1=ALU.add)
                    else:
                        xt_new = xt
                    x, xt = x_new, xt_new
                return xt if trans else x

            def renorm_update(wn, wt, dwx, nn, nmt):
                """wn <- (wn + dwx) / (||row|| + 1e-5) * nn; rebuild wt."""
                nf = wn.shape[2]
                tmp = work.tile([128, 1024], f32, tag="junkw")
                tmpw = tmp[:, 0:nmt * nf].rearrange("p (a b) -> p a b", a=nmt)
                nc.vector.tensor_add(tmpw, wn.bitcast(f32), dwx.bitcast(f32))
                rs = small.tile([128, 4], f32, tag="rs")
                for mt in range(nmt):
                    junk = work.tile([128, 1024], f32, tag="junkw")
                    nc.scalar.activation(out=junk[:, 0:nf], in_=tmpw[:, mt, :],
                                         func=AF.Square, accum_out=rs[:, mt:mt + 1])
                nc.scalar.activation(out=rs[:, 0:nmt], in_=rs[:, 0:nmt], func=AF.Sqrt)
                nc.vector.tensor_scalar_add(rs[:, 0:nmt], rs[:, 0:nmt], 1e-5)
                nc.vector.reciprocal(rs[:, 0:nmt], rs[:, 0:nmt])
                rsc = small.tile([128, 4], f32, tag="rsc")
                nc.vector.tensor_mul(rsc[:, 0:nmt], rs[:, 0:nmt], nn)
                for mt in range(nmt):
                    nc.vector.tensor_scalar_mul(wn[:, mt, :], tmpw[:, mt, :],
                                                rsc[:, mt:mt + 1])
                build_T(wn, wt, nmt)

            # ================= main chunk loop =================
            parts = os.environ.get("KPARTS", "all")
            for ci in range(n_chunks):
                nc.vector.memset(dw0a, 0.0)
                nc.vector.memset(dw1a, 0.0)
                nc.vector.memset(dw2a, 0.0)
                nc.vector.memset(oc, 0.0)
                for hf in range(2):
                    gtb_base = ci * 16 + hf * 8
                    if parts != "none":
                        phase1(gtb_base)
                    if parts in ("all", "p2a", "p2ab"):
                        phase2a(gtb_base)
                    if parts in ("all", "p2b", "p2ab"):
                        phase2b()
                    nc.sync.dma_start(out=out_r[:, gtb_base:gtb_base + 8, :], in_=oc)

                if parts == "all":
                    dw0x = newton_schulz(dw0a, trans=True)
                    renorm_update(w0n, w0t, dw0x, n0, 4)
                    dw1x = newton_schulz(dw1a, trans=False)
                    renorm_update(w1n, w1t, dw1x, n1, 2)
                    dw2x = newton_schulz(dw2a, trans=True)
                    renorm_update(w2n, w2t, dw2x, n2, 4)

    nc.compile()
    return nc


_NC_CACHE = {}


def _get_nc(s_total):
    if s_total not in _NC_CACHE:
        _NC_CACHE[s_total] = build_nc(s_total)
    return _NC_CACHE[s_total]


def make_in_maps(hidden_states, qkv_w, q_norm_w, k_norm_w, lr_proj_w, lr_proj_b,
                 w0, w1, w2, s_total=S):
    hidden_states = np.asarray(hidden_states, dtype=np.float32)
    qkv_w = np.asarray(qkv_w, dtype=np.float32)
    q_norm_w = np.asarray(q_norm_w, dtype=np.float32)
    k_norm_w = np.asarray(k_norm_w, dtype=np.float32)
    lr_proj_w = np.asarray(lr_proj_w, dtype=np.float32)
    lr_proj_b = np.asarray(lr_proj_b, dtype=np.float32)
    w0 = np.asarray(w0, dtype=np.float32)
    w1 = np.asarray(w1, dtype=np.float32)
    w2 = np.asarray(w2, dtype=np.float32)

    ident = np.eye(128, dtype=np.float32)
    ones_c = np.ones((128, 1), np.float32)
    ones_r = np.ones((1, 128), np.float32)
    in_maps = []
    for core in range(NCORES):
        b, h = divmod(core, NH)
        hs = h * DH
        wcat = np.zeros((D, WCOLS), np.float32)
        wcat[:, 0:256] = (qkv_w[hs:hs + DH, :] * q_norm_w[hs:hs + DH, None]).T
        wcat[:, 256:512] = (qkv_w[D + hs:D + hs + DH, :] * k_norm_w[hs:hs + DH, None]).T
        wcat[:, 512:768] = qkv_w[2 * D + hs:2 * D + hs + DH, :].T
        wcat[:, 768:771] = lr_proj_w[3 * h:3 * h + 3, :].T
        lrb = (lr_proj_b[3 * h:3 * h + 3] + BASE_LR_INV).reshape(1, 3)
        in_maps.append({
            "hidden": np.ascontiguousarray(hidden_states[b, :s_total]),
            "wcat": wcat,
            "lrb": lrb,
            "w0": w0[h],
            "w1": w1[h],
            "w2": w2[h],
            "ident": ident,
            "ones_c": ones_c,
            "ones_r": ones_r,
        })
    return in_maps


def kernel(hidden_states, qkv_w, q_norm_w, k_norm_w, lr_proj_w, lr_proj_b,
           w0, w1, w2, s_total=S):
    nc = _get_nc(s_total)
    in_maps = make_in_maps(hidden_states, qkv_w, q_norm_w, k_norm_w,
                           lr_proj_w, lr_proj_b, w0, w1, w2, s_total)
    res = run_bass_kernel_spmd(nc, in_maps, core_ids=list(range(NCORES)))
    out = np.empty((B, s_total, D), np.float32)
    for core in range(NCORES):
        b, h = divmod(core, NH)
        out[b, :, h * DH:(h + 1) * DH] = res.results[core]["out"]
    return out
